# revision 6
# baseline (speedup 1.0000x reference)
"""Corr1d (stereo cost volume) Trainium2 kernel, v5.

corrmap[b, i, h, w] = sum_c fL[b, c, h, w] * fR[b, c, h, w - i],  i in [0, 64)
Shapes: fL, fR [8, 128, 160, 320] f32 -> corrmap [8, 64, 160, 320] f32.
Sharding: data-parallel over batch; core k handles batch element k.

v5 vs v4: the previous half's shear work is interleaved into the next
half's Gram chunk stream so the in-order PE never idles at the half
boundary; Gslab double-buffered (shear(q-1) reads while Gram(q)
writes), outstage single-buffered (its output DMA drains a full half
before the next shear).

Self-contained: requires only numpy + ml_dtypes + concourse.
"""

import numpy as np
import ml_dtypes

import concourse.bacc as bacc
import concourse.bass as bass
import concourse.mybir as mybir
from concourse.bass_utils import run_bass_kernel_spmd
from concourse.tile import TileContext

F32 = mybir.dt.float32
BF16 = mybir.dt.bfloat16

N_CORES = 8
C = 128
H = 160
W = 320
D = 64
HW = H * W
NQ = 2            # h halves
HC = H // NQ      # 80 rows per half
NCK = 10          # input chunks per half
RK = HC // NCK    # 8 rows per chunk
NS = W // D       # 5 w-tiles of 64
QROW = NS * HC * D  # Gslab row length (elements per partition)

# shear pair families: (top_s, top_j0, bot_s, bot_j0); t in [0,32),
# top w = 64*ts + tj0 + t, bottom w = top w + 160.
FAMS = [(0, 0, 2, 32), (0, 32, 3, 0), (1, 0, 3, 32), (1, 32, 4, 0),
        (2, 0, 4, 32)]
# j-groups within a family: (group start, total nj, nj per psum bank)
JGROUPS = [(0, 12, 6), (12, 12, 6), (24, 8, 4)]

_cache = {}


def _shift_consts():
    sh = np.zeros((128, 64 * 64), np.float32)
    sh0 = np.zeros((64, 64 * 64), np.float32)
    for j in range(64):
        for m in range(64):
            sh[64 + j - m, 64 * j + m] = 1.0
            if 0 <= j - m:
                sh0[j - m, 64 * j + m] = 1.0
    return sh.astype(ml_dtypes.bfloat16), sh0.astype(ml_dtypes.bfloat16)


def _build():
    nc = bacc.Bacc("TRN2", target_bir_lowering=False, debug=False,
                   num_devices=N_CORES)
    fL = nc.dram_tensor("fL", [C, H, W], BF16, kind="ExternalInput")
    fR = nc.dram_tensor("fR", [C, H, W], BF16, kind="ExternalInput")
    out = nc.dram_tensor("out", [D, H, W], BF16, kind="ExternalOutput")
    shnp, sh0np = _shift_consts()
    shd = nc.inline_tensor(shnp, name="shd")
    sh0d = nc.inline_tensor(sh0np, name="sh0d")

    with TileContext(nc) as tc:
        sh = nc.alloc_sbuf_tensor("sh", [128, 64 * 64], BF16)
        sh0 = nc.alloc_sbuf_tensor("sh0", [64, 64 * 64], BF16)
        nc.scalar.dma_start(
            out=sh.ap(), in_=bass.AP(shd, 0, [[64 * 64, 128], [1, 64 * 64]]))
        nc.scalar.dma_start(
            out=sh0.ap(), in_=bass.AP(sh0d, 0, [[64 * 64, 64], [1, 64 * 64]]))

        gslab = [nc.alloc_sbuf_tensor(f"gslab{i}", [C, QROW], BF16)
                 for i in range(2)]
        ost = [nc.alloc_sbuf_tensor(f"ost{i}", [C, HC * 160], BF16)
               for i in range(2)]

        cp_state = [0]

        def copy(o, i_):
            e = cp_state[0] % 2
            cp_state[0] += 1
            if e == 0:
                nc.vector.tensor_copy(out=o, in_=i_)
            else:
                nc.scalar.copy(o, i_)

        with (
            tc.tile_pool(name="sb", bufs=3) as pool,
            tc.tile_pool(name="ps", bufs=3, space="PSUM") as pp,
        ):
            def emit_chunk(q, ck):
                gs = gslab[q % 2]
                r0 = q * HC + ck * RK
                fLc = pool.tile([C, RK * W], BF16, tag="fL",
                                name=f"fL_{q}_{ck}")
                fRc = pool.tile([C, RK * W], BF16, tag="fR",
                                name=f"fR_{q}_{ck}")
                qL = nc.sync if ck % 2 == 0 else nc.scalar
                qR = nc.gpsimd
                qL.dma_start(
                    out=fLc, in_=bass.AP(fL, r0 * W, [[HW, C], [1, RK * W]]))
                qR.dma_start(
                    out=fRc, in_=bass.AP(fR, r0 * W, [[HW, C], [1, RK * W]]))
                for s in range(NS):
                    M = 64 if s == 0 else 128
                    w0 = max(64 * s - 64, 0)
                    ps = pp.tile([128, 512], F32, tag="g", bufs=4,
                                 name=f"g_{q}_{ck}_{s}")
                    for r in range(RK):
                        b = r * W
                        nc.tensor.matmul(
                            ps[:M, r * D:(r + 1) * D],
                            fRc[:, b + w0: b + w0 + M],
                            fLc[:, b + 64 * s: b + 64 * s + D],
                            start=True, stop=True,
                        )
                    copy(
                        bass.AP(gs, s * HC * D + ck * RK * D,
                                [[QROW, M], [1, RK * D]]),
                        bass.AP(ps.tensor, 0, [[512, M], [1, RK * D]]),
                    )

            def emit_shear_item(q, item):
                gs = gslab[q % 2]
                ts, tj0, bs, bj0, g0, nj, njb = item
                ps2 = pp.tile([128, 1024], F32, tag="sh",
                              name=f"sh_{q}_{ts}_{tj0}_{g0}", bufs=2)
                for t in range(nj):
                    tt = g0 + t
                    co = t % njb + 512 * (t // njb)
                    jt = tj0 + tt
                    shin = sh0 if ts == 0 else sh
                    K = 64 if ts == 0 else 128
                    nc.tensor.matmul(
                        bass.AP(ps2.tensor, co, [[1024, 64], [njb, HC]]),
                        shin.ap()[:, D * jt: D * jt + D],
                        bass.AP(gs, ts * HC * D + jt, [[QROW, K], [D, HC]]),
                        start=True, stop=True,
                    )
                    jb = bj0 + tt
                    nc.tensor.matmul(
                        bass.AP(ps2.tensor, 64 * 1024 + co,
                                [[1024, 64], [njb, HC]]),
                        sh.ap()[:, D * jb: D * jb + D],
                        bass.AP(gs, bs * HC * D + jb, [[QROW, 128], [D, HC]]),
                        start=True, stop=True,
                    )
                copy(
                    bass.AP(ost[q % 2], 64 * ts + tj0 + g0,
                            [[HC * 160, 128], [160, HC],
                             [njb, 2], [1, njb]]),
                    bass.AP(ps2.tensor, 0,
                            [[1024, 128], [njb, HC], [512, 2], [1, njb]]),
                )

            def emit_out_dma(q):
                h0 = q * HC
                nc.sync.dma_start(
                    out=bass.AP(out, h0 * W, [[HW, D], [W, HC], [1, 160]]),
                    in_=bass.AP(ost[q % 2], 0,
                                [[HC * 160, D], [160, HC], [1, 160]]),
                )
                nc.sync.dma_start(
                    out=bass.AP(out, h0 * W + 160,
                                [[HW, D], [W, HC], [1, 160]]),
                    in_=bass.AP(ost[q % 2], 64 * HC * 160,
                                [[HC * 160, D], [160, HC], [1, 160]]),
                )

            def emit_out_dma_w(q, wtop, ncols):
                # store w-cols [wtop, wtop+ncols) and [wtop+160, ...) of
                # half q, overlapping the tail shear
                h0 = q * HC
                nc.sync.dma_start(
                    out=bass.AP(out, h0 * W + wtop,
                                [[HW, D], [W, HC], [1, ncols]]),
                    in_=bass.AP(ost[q % 2], wtop,
                                [[HC * 160, D], [160, HC], [1, ncols]]),
                )
                nc.sync.dma_start(
                    out=bass.AP(out, h0 * W + wtop + 160,
                                [[HW, D], [W, HC], [1, ncols]]),
                    in_=bass.AP(ost[q % 2], 64 * HC * 160 + wtop,
                                [[HC * 160, D], [160, HC], [1, ncols]]),
                )

            items = [(ts, tj0, bs, bj0, g0, nj, njb)
                     for ts, tj0, bs, bj0 in FAMS
                     for g0, nj, njb in JGROUPS]

            for ck in range(NCK):
                emit_chunk(0, ck)
            for q in range(1, NQ):
                # interleave previous half's shear into this half's chunks
                done = 0
                for ck in range(NCK):
                    emit_chunk(q, ck)
                    want = (ck + 1) * len(items) // NCK
                    while done < want:
                        emit_shear_item(q - 1, items[done])
                        done += 1
                emit_out_dma(q - 1)
            for idx, it in enumerate(items):
                emit_shear_item(NQ - 1, it)
            # adjacent-w family pairs -> 128B-row stores; emitted after the
            # covering items so each store overlaps the remaining tail work
            # (f0+f1: w 0-63/160-223 after item 5; f2+f3 after item 11;
            #  f4 after item 14 — all items already emitted above, so the
            #  dependency tracker orders them; issue order still matters)
            emit_out_dma_w(NQ - 1, 0, 64)
            emit_out_dma_w(NQ - 1, 64, 64)
            emit_out_dma_w(NQ - 1, 128, 32)

    nc.compile()
    return nc


def kernel(fL: np.ndarray, fR: np.ndarray) -> np.ndarray:
    if "nc" not in _cache:
        _cache["nc"] = _build()
    nc = _cache["nc"]

    fLb = np.ascontiguousarray(
        np.asarray(fL, dtype=np.float32)).astype(ml_dtypes.bfloat16)
    fRb = np.ascontiguousarray(
        np.asarray(fR, dtype=np.float32)).astype(ml_dtypes.bfloat16)
    in_maps = [{"fL": fLb[k], "fR": fRb[k]} for k in range(N_CORES)]
    res = run_bass_kernel_spmd(nc, in_maps, core_ids=list(range(N_CORES)))
    outs = [np.asarray(res.results[k]["out"]).astype(np.float32)
            for k in range(N_CORES)]
    return np.stack(outs, axis=0)


if __name__ == "__main__":
    rng = np.random.default_rng(0)
    a = rng.standard_normal((N_CORES, C, H, W)).astype(np.float32)
    b = rng.standard_normal((N_CORES, C, H, W)).astype(np.float32)
    o = kernel(a, b)
    print("kernel ran, output shape", o.shape, o.dtype)
